# revision 7
# baseline (speedup 1.0000x reference)
"""Bass/Tile Trainium2 kernel for the additive-attention module (v2).

reference (per batch row b):
    q = hidden_state @ Wa.T + ba                 # [A]
    k = feature_vectors[b] @ Ua.T                # [L, A]
    e = tanh(q + k) @ w                          # [L]
    attn = softmax(e)                            # [L]
    context[b] = attn @ feature_vectors[b]       # [M]

Sharding: data-parallel over batch B=64 -> 8 cores x 8 rows, params
replicated, no collectives.

v2 dataflow: the host pre-transposes + fp16-casts feature_vectors to
fvT [B, M, L] (a storage-layout choice made once, off the device), so
each core streams 16 MB of fp16 from HBM in 8 KB-contiguous
descriptors and the PE never transposes fv:
  - k-matmul [a, l]: UaT chunks stationary, fvT columns streaming
    (fp16, 512-col streams, weight reuse order: 16 loads/row)
  - ScalarE evacuates k PSUM with fused per-partition bias q[a] + tanh
  - e = w.T @ t via 1-column matmuls with t chunks stationary,
    accumulated directly into a [128, 32] PSUM tile that IS the
    softmax layout (e[l] at partition l%128, column l//128)
  - softmax: DVE row-max, GPSIMD cross-partition max, ACT exp with
    accumulated row sums, GPSIMD cross-partition sum
  - weighted sum on DVE: p bounced through DRAM (8 KB) into a
    [128, L] partition-broadcast, then one fused multiply+reduce
    (tensor_tensor_reduce) per m-half against fvT, scaled by 1/Z
  - per-row stages software-pipelined: row b's p-chain is emitted
    inside row b+1's PE stream, weighted sum trails by two rows
"""

import numpy as np

B, R, M, A, L = 64, 512, 256, 256, 4096
NCORES = 8
BLOC = B // NCORES  # 8 batch rows per core
NJG = 4  # j-groups of 1024 l-columns
JGW = L // NJG  # 1024
JW2 = JGW // 2  # 512 (psum bank width in f32)
NL = L // 128  # 32 e-columns

_CACHE = {}


def _build():
    from contextlib import ExitStack

    import concourse.bacc as bacc
    import concourse.bass as bass
    import concourse.bass_isa as bass_isa
    import concourse.mybir as mybir
    import concourse.tile as tile
    from concourse.masks import make_identity

    f32 = mybir.dt.float32
    f16 = mybir.dt.float16
    AF = mybir.ActivationFunctionType
    ALU = mybir.AluOpType

    nc = bacc.Bacc("TRN2", target_bir_lowering=False, debug=False,
                   num_devices=NCORES)

    hs = nc.dram_tensor("hidden_state", [BLOC, R], f32, kind="ExternalInput").ap()
    fvt_d = nc.dram_tensor("fvT", [BLOC, M, L], f16, kind="ExternalInput").ap()
    Wa = nc.dram_tensor("Wa", [A, R], f32, kind="ExternalInput").ap()
    Ua = nc.dram_tensor("Ua", [A, M], f32, kind="ExternalInput").ap()
    w = nc.dram_tensor("w", [A, 1], f32, kind="ExternalInput").ap()
    ba = nc.dram_tensor("ba", [1, A], f32, kind="ExternalInput").ap()
    ctx_out = nc.dram_tensor("context", [BLOC, M], f32, kind="ExternalOutput").ap()

    with tile.TileContext(nc) as tc, ExitStack() as ctx:
        singles = ctx.enter_context(tc.tile_pool(name="singles", bufs=1))
        ldpool = ctx.enter_context(tc.tile_pool(name="ldpool", bufs=2))
        fvpool = ctx.enter_context(tc.tile_pool(name="fvpool", bufs=4))
        tpool = ctx.enter_context(tc.tile_pool(name="tpool", bufs=3))
        bcpool = ctx.enter_context(tc.tile_pool(name="bcpool", bufs=2))
        trashp = ctx.enter_context(tc.tile_pool(name="trashp", bufs=2))
        ptpool = ctx.enter_context(tc.tile_pool(name="ptpool", bufs=3))
        small = ctx.enter_context(tc.tile_pool(name="small", bufs=3))
        ps_k = ctx.enter_context(tc.tile_pool(name="ps_k", bufs=2, space="PSUM"))
        ps_e = ctx.enter_context(tc.tile_pool(name="ps_e", bufs=2, space="PSUM"))
        ps_sm = ctx.enter_context(tc.tile_pool(name="ps_sm", bufs=2, space="PSUM"))
        dram = ctx.enter_context(tc.tile_pool(name="dram", bufs=3, space="DRAM"))

        # batch row 0's fvT load first (in two halves so jG 0/1 can start
        # as soon as the first MB lands)
        fvt0 = fvpool.tile([128, 2, L], f16, tag="fvt", name="fvt")
        for half in range(2):
            src = bass.AP(tensor=fvt_d.tensor, offset=half * (L // 2),
                          ap=[[L, 128], [128 * L, 2], [1, L // 2]])
            nc.gpsimd.dma_start(
                out=fvt0[:, :, half * (L // 2):(half + 1) * (L // 2)], in_=src)

        ident = singles.tile([128, 128], f32, tag="ident", name="ident")
        make_identity(nc, ident)
        ident16 = singles.tile([128, 128], f16, tag="ident16", name="ident16")
        make_identity(nc, ident16)

        # ---- parameters into contraction-major layouts ----
        # WaT[rt] [128(r), 256(a)] fp32: WaT[rt][k, a] = Wa[a, 128*rt + k]
        WaT = [singles.tile([128, A], f32, tag=f"WaT{rt}", name=f"WaT{rt}")
               for rt in range(4)]
        for at in range(2):
            wa_nat = ldpool.tile([128, R], f32, tag="ld", name="ld")
            nc.sync.dma_start(out=wa_nat, in_=Wa[at * 128:(at + 1) * 128, :])
            for rt in range(4):
                ps = ps_sm.tile([128, 128], f32, tag="sm", name="sm")
                nc.tensor.transpose(ps, wa_nat[:, rt * 128:(rt + 1) * 128], ident)
                nc.vector.tensor_copy(out=WaT[rt][:, at * 128:(at + 1) * 128],
                                      in_=ps)
        # UaT[mh] [128(m), 256(a)] fp16: UaT[mh][k, a] = Ua[a, 128*mh + k]
        UaT = [singles.tile([128, A], f16, tag=f"UaT{mh}", name=f"UaT{mh}")
               for mh in range(2)]
        for at in range(2):
            ua_nat = ldpool.tile([128, M], f32, tag="ld", name="ld")
            nc.sync.dma_start(out=ua_nat, in_=Ua[at * 128:(at + 1) * 128, :])
            for mh in range(2):
                ps = ps_sm.tile([128, 128], f32, tag="sm", name="sm")
                nc.tensor.transpose(ps, ua_nat[:, mh * 128:(mh + 1) * 128], ident)
                nc.vector.tensor_copy(out=UaT[mh][:, at * 128:(at + 1) * 128],
                                      in_=ps)
        # w as fp16 stationary columns [128, 1] per a-half (cast during DMA)
        w_sb = [singles.tile([128, 1], f16, tag=f"w{ah}", name=f"w{ah}")
                for ah in range(2)]
        for ah in range(2):
            nc.gpsimd.dma_start(out=w_sb[ah], in_=w[ah * 128:(ah + 1) * 128, :])

        # hsT[rt] [128(r), BLOC] fp32
        hsT = [singles.tile([128, BLOC], f32, tag=f"hsT{rt}", name=f"hsT{rt}")
               for rt in range(4)]
        for rt in range(4):
            src = bass.AP(tensor=hs.tensor, offset=rt * 128,
                          ap=[[1, 128], [R, BLOC]])
            nc.sync.dma_start(out=hsT[rt], in_=src)

        # q = hs @ Wa.T + ba   -> [BLOC, A] fp32
        q_ps = ps_sm.tile([BLOC, A], f32, tag="sm", name="sm")
        for rt in range(4):
            nc.tensor.matmul(q_ps, lhsT=hsT[rt], rhs=WaT[rt],
                             start=(rt == 0), stop=(rt == 3))
        ba_b = singles.tile([BLOC, A], f32, tag="ba", name="ba")
        nc.sync.dma_start(out=ba_b,
                          in_=bass.AP(tensor=ba.tensor, offset=0,
                                      ap=[[0, BLOC], [1, A]]))
        q_sb = singles.tile([BLOC, A], f32, tag="q", name="q")
        nc.vector.tensor_add(q_sb, q_ps, ba_b)
        # qT[ah] [128(a), BLOC] fp32
        qT = [singles.tile([128, BLOC], f32, tag=f"qT{ah}", name=f"qT{ah}")
              for ah in range(2)]
        for ah in range(2):
            ps = ps_sm.tile([128, BLOC], f32, tag="sm", name="sm")
            nc.tensor.transpose(ps, q_sb[:, ah * 128:(ah + 1) * 128],
                                ident[:BLOC, :BLOC])
            nc.vector.tensor_copy(out=qT[ah], in_=ps)

        # ---- per-batch-row pipeline ----
        state = {}  # b -> dict(fvt, psE, t_q, p_t, rz, p_bc)

        def emit_load(b):
            if b == 0:
                fvt = fvt0
            else:
                fvt = fvpool.tile([128, 2, L], f16, tag="fvt", name="fvt")
                src = bass.AP(tensor=fvt_d.tensor, offset=b * M * L,
                              ap=[[L, 128], [128 * L, 2], [1, L]])
                nc.gpsimd.dma_start(out=fvt, in_=src)
            st = {"fvt": fvt, "t_q": {}}
            st["e_sb"] = small.tile([1, L], f32, tag="e_sb", name="e_sb")
            st["e_d"] = dram.tile([L], f32, tag="e_d", name="e_d")
            state[b] = st

        def emit_k(b, jg):
            st = state[b]
            ts = []
            for ah in range(2):
                psk = ps_k.tile([128, 2, JW2], f32, tag="psk", name="psk")
                for mh in range(2):
                    for c in range(2):
                        lo = jg * JGW + c * JW2
                        nc.tensor.matmul(
                            psk[:, c, :],
                            lhsT=UaT[mh][:, ah * 128:(ah + 1) * 128],
                            rhs=st["fvt"][:, mh, lo:lo + JW2],
                            start=(mh == 0), stop=(mh == 1))
                t_sb = tpool.tile([128, 2, JW2], f16, tag=f"t{ah}",
                                  name=f"t{ah}")
                nc.scalar.activation(out=t_sb, in_=psk, func=AF.Tanh,
                                     bias=qT[ah][:, b:b + 1], scale=1.0)
                ts.append(t_sb)
            st["t_q"][jg] = ts

        def emit_e(b, jg):
            st = state[b]
            ts = st["t_q"].pop(jg)
            e_sb = st["e_sb"]
            for c in range(2):
                pse = ps_e.tile([1, JW2], f32, tag="ee", name="ee")
                for ah in range(2):
                    nc.tensor.matmul(pse, lhsT=w_sb[ah], rhs=ts[ah][:, c, :],
                                     start=(ah == 0), stop=(ah == 1))
                lo = jg * JGW + c * JW2
                if c == 0:
                    nc.vector.tensor_copy(out=e_sb[:, lo:lo + JW2], in_=pse)
                else:
                    nc.scalar.copy(out=e_sb[:, lo:lo + JW2], in_=pse)
            if jg == NJG - 1:
                nc.sync.dma_start(
                    out=bass.AP(tensor=st["e_d"].tensor, offset=st["e_d"].offset,
                                ap=[[0, 1], [1, L]]),
                    in_=e_sb)

        def emit_softmax(b):
            st = state[b]
            e_t = small.tile([128, NL], f32, tag="e_t", name="e_t")
            nc.sync.dma_start(
                out=e_t,
                in_=bass.AP(tensor=st["e_d"].tensor, offset=st["e_d"].offset,
                            ap=[[1, 128], [128, NL]]))
            st["psE"] = e_t
            mrow = small.tile([128, 1], f32, tag="mrow", name="mrow")
            nc.vector.reduce_max(out=mrow, in_=e_t, axis=mybir.AxisListType.X)
            mall = small.tile([128, 1], f32, tag="mall", name="mall")
            nc.gpsimd.partition_all_reduce(mall, mrow, channels=128,
                                           reduce_op=bass_isa.ReduceOp.max)
            negm = small.tile([128, 1], f32, tag="negm", name="negm")
            nc.vector.tensor_scalar_mul(negm, mall, -1.0)
            p_t = ptpool.tile([128, NL], f16, tag="p_t", name="p_t")
            srow = small.tile([128, 1], f32, tag="srow", name="srow")
            nc.scalar.activation(out=p_t, in_=e_t, func=AF.Exp, bias=negm,
                                 scale=1.0, accum_out=srow)
            sall = small.tile([128, 1], f32, tag="sall", name="sall")
            nc.gpsimd.partition_all_reduce(sall, srow, channels=128,
                                           reduce_op=bass_isa.ReduceOp.add)
            rz = small.tile([128, 1], f32, tag="rz", name="rz")
            nc.vector.reciprocal(out=rz, in_=sall)
            st["p_t"] = p_t
            st["rz"] = rz

        def emit_pchain(b):
            # p [128, 32] -> [32, 128] -> DRAM row -> [128, L] broadcast
            st = state[b]
            pT_ps = ps_sm.tile([32, 128], f16, tag="sm", name="sm")
            nc.tensor.transpose(pT_ps, st["p_t"], ident16)
            pT_sb = small.tile([32, 128], f16, tag="ptsb", name="ptsb")
            nc.vector.tensor_copy(out=pT_sb, in_=pT_ps)
            p_d = dram.tile([L], f16, tag="p_d", name="p_d")
            nc.sync.dma_start(
                out=bass.AP(tensor=p_d.tensor, offset=p_d.offset,
                            ap=[[128, 32], [1, 128]]),
                in_=pT_sb)
            p_bc = bcpool.tile([128, L], f16, tag="p_bc", name="p_bc")
            nc.sync.dma_start(
                out=p_bc,
                in_=bass.AP(tensor=p_d.tensor, offset=p_d.offset,
                            ap=[[0, 128], [1, L]]))
            st["p_bc"] = p_bc

        def emit_ws(b):
            st = state.pop(b)
            for mh in range(2):
                prod = trashp.tile([128, L], f16, tag="trash", name="trash")
                nc.vector.tensor_mul(prod, st["p_bc"], st["fvt"][:, mh, :])
                s1 = trashp.tile([128, L // 2], f16, tag="s1", name="s1")
                nc.vector.tensor_add(s1, prod[:, :L // 2], prod[:, L // 2:])
                s2 = trashp.tile([128, L // 4], f16, tag="s2", name="s2")
                nc.vector.tensor_add(s2, s1[:, :L // 4], s1[:, L // 4:])
                s3 = trashp.tile([128, L // 8], f16, tag="s3", name="s3")
                nc.vector.tensor_add(s3, s2[:, :L // 8], s2[:, L // 8:])
                ctxh = small.tile([128, 1], f32, tag=f"ctxh{mh}",
                                  name=f"ctxh{mh}")
                nc.vector.tensor_reduce(out=ctxh, in_=s3,
                                        axis=mybir.AxisListType.X, op=ALU.add)
                ctxs = small.tile([128, 1], f32, tag=f"ctxs{mh}",
                                  name=f"ctxs{mh}")
                nc.vector.tensor_mul(ctxs, ctxh, st["rz"])
                nc.sync.dma_start(
                    out=bass.AP(tensor=ctx_out.tensor, offset=b * M + mh * 128,
                                ap=[[1, 128], [0, 1]]),
                    in_=ctxs)

        for b in range(BLOC):
            emit_load(b)
            emit_k(b, 0)
            emit_k(b, 1)
            if b >= 1:
                emit_pchain(b - 1)
            emit_e(b, 0)
            emit_k(b, 2)
            emit_e(b, 1)
            emit_k(b, 3)
            emit_e(b, 2)
            emit_e(b, 3)
            if b >= 2:
                emit_ws(b - 2)
            emit_softmax(b)
        emit_pchain(BLOC - 1)
        emit_ws(BLOC - 2)
        emit_ws(BLOC - 1)

    nc.compile()
    return nc


def _get_nc():
    if "nc" not in _CACHE:
        _CACHE["nc"] = _build()
    return _CACHE["nc"]


def make_in_maps(inputs):
    """Per-core input dicts for run_bass_kernel_spmd (host-side shard +
    fp16 pre-transpose of feature_vectors)."""
    fv = np.asarray(inputs["feature_vectors"])
    fvT = fv.transpose(0, 2, 1).astype(np.float16)  # [B, M, L] fp16
    hs = np.ascontiguousarray(np.asarray(inputs["hidden_state"]),
                              dtype=np.float32)
    params = {
        "Wa": np.ascontiguousarray(np.asarray(inputs["Wa"]), dtype=np.float32),
        "Ua": np.ascontiguousarray(np.asarray(inputs["Ua"]), dtype=np.float32),
        "w": np.ascontiguousarray(np.asarray(inputs["w"]), dtype=np.float32),
        "ba": np.ascontiguousarray(np.asarray(inputs["ba"]), dtype=np.float32),
    }
    return [
        {
            "hidden_state": hs[c * BLOC:(c + 1) * BLOC],
            "fvT": np.ascontiguousarray(fvT[c * BLOC:(c + 1) * BLOC]),
            **params,
        }
        for c in range(NCORES)
    ]


def kernel(hidden_state, feature_vectors, Wa, Ua, w, ba):
    from concourse.bass_utils import run_bass_kernel_spmd

    nc = _get_nc()
    in_maps = make_in_maps({
        "hidden_state": hidden_state,
        "feature_vectors": feature_vectors,
        "Wa": Wa, "Ua": Ua, "w": w, "ba": ba,
    })
    res = run_bass_kernel_spmd(nc, in_maps, list(range(NCORES)))
    return np.concatenate([res.results[c]["context"] for c in range(NCORES)],
                          axis=0)


# revision 8
# speedup vs baseline: 1.1406x; 1.1406x over previous
"""Bass/Tile Trainium2 kernel for the additive-attention module (v5).

reference (per batch row b):
    q = hidden_state @ Wa.T + ba                 # [A]
    k = feature_vectors[b] @ Ua.T                # [L, A]
    e = tanh(q + k) @ w                          # [L]
    attn = softmax(e)                            # [L]
    context[b] = attn @ feature_vectors[b]       # [M]

Sharding: data-parallel over batch B=64 -> 8 cores x 8 rows, params
replicated, no collectives. Host pre-transposes + fp16-casts
feature_vectors to fvT [B, M, L] once, off-device.

v5 = flat cross-row software pipeline tuned from the v4 HW trace:
  - PE stream per row: k(jG) 512-col fp16 matmuls with UaT stationary,
    w-stationary e-row matmuls lagging their jG by 2 (they also cover
    the psk-recycle latency), row b-1's tail e-matmuls and its p_t
    transpose embedded in row b's stream so PE never drains.
  - Act: 2-bank tanh ACTIVATE straight from PSUM with per-partition
    q bias; ALL pse evacuations (e rows) run on Act; exp of row b-1
    lands mid-row-b between tanh groups.
  - softmax chain spread across engines so no in-order queue blocks:
    DVE row-max -> gpsimd cross-partition max -> gpsimd negate ->
    Act exp(+accum row sums) -> gpsimd sum -> DVE reciprocal (late).
  - weighted sum on DVE: fp16 multiply (2x mode) + add-tree to 512 +
    one short reduce per m-half; p broadcast [128, L] via SWDGE DMA
    (HWDGE chokes ~50ns/descriptor; SWDGE posts the 128x8KB fan-out
    in ~1.1us).
  - e-row bounce e_sb -> DRAM -> [128, 32] on the sync queue (small,
    2D APs only).
"""

import numpy as np

B, R, M, A, L = 64, 512, 256, 256, 4096
NCORES = 8
BLOC = B // NCORES  # 8 batch rows per core
NJG = 4  # j-groups of 1024 l-columns
JGW = L // NJG  # 1024
JW2 = JGW // 2  # 512 (psum bank width in f32)
NL = L // 128  # 32 e-columns

_CACHE = {}


def _build():
    from contextlib import ExitStack

    import concourse.bacc as bacc
    import concourse.bass as bass
    import concourse.bass_isa as bass_isa
    import concourse.mybir as mybir
    import concourse.tile as tile
    from concourse.masks import make_identity

    f32 = mybir.dt.float32
    f16 = mybir.dt.float16
    AF = mybir.ActivationFunctionType
    ALU = mybir.AluOpType

    nc = bacc.Bacc("TRN2", target_bir_lowering=False, debug=False,
                   num_devices=NCORES)

    hs = nc.dram_tensor("hidden_state", [BLOC, R], f32, kind="ExternalInput").ap()
    fvt_d = nc.dram_tensor("fvT", [BLOC, M, L], f16, kind="ExternalInput").ap()
    Wa = nc.dram_tensor("Wa", [A, R], f32, kind="ExternalInput").ap()
    Ua = nc.dram_tensor("Ua", [A, M], f32, kind="ExternalInput").ap()
    w = nc.dram_tensor("w", [A, 1], f32, kind="ExternalInput").ap()
    ba = nc.dram_tensor("ba", [1, A], f32, kind="ExternalInput").ap()
    ctx_out = nc.dram_tensor("context", [BLOC, M], f32, kind="ExternalOutput").ap()

    with tile.TileContext(nc) as tc, ExitStack() as ctx:
        singles = ctx.enter_context(tc.tile_pool(name="singles", bufs=1))
        ldpool = ctx.enter_context(tc.tile_pool(name="ldpool", bufs=2))
        fvpool = ctx.enter_context(tc.tile_pool(name="fvpool", bufs=4))
        tpool = ctx.enter_context(tc.tile_pool(name="tpool", bufs=4))
        bcpool = ctx.enter_context(tc.tile_pool(name="bcpool", bufs=2))
        trashp = ctx.enter_context(tc.tile_pool(name="trashp", bufs=2))
        ptpool = ctx.enter_context(tc.tile_pool(name="ptpool", bufs=3))
        small = ctx.enter_context(tc.tile_pool(name="small", bufs=3))
        ps_k = ctx.enter_context(tc.tile_pool(name="ps_k", bufs=2, space="PSUM"))
        ps_e = ctx.enter_context(tc.tile_pool(name="ps_e", bufs=2, space="PSUM"))
        ps_sm = ctx.enter_context(tc.tile_pool(name="ps_sm", bufs=2, space="PSUM"))
        dram = ctx.enter_context(tc.tile_pool(name="dram", bufs=3, space="DRAM"))

        # row 0's fvT load first, in two halves
        fvt0 = fvpool.tile([128, 2, L], f16, tag="fvt", name="fvt")
        for half in range(2):
            src = bass.AP(tensor=fvt_d.tensor, offset=half * (L // 2),
                          ap=[[L, 128], [128 * L, 2], [1, L // 2]])
            nc.gpsimd.dma_start(
                out=fvt0[:, :, half * (L // 2):(half + 1) * (L // 2)], in_=src)

        ident = singles.tile([128, 128], f32, tag="ident", name="ident")
        make_identity(nc, ident)
        ident16 = singles.tile([128, 128], f16, tag="ident16", name="ident16")
        make_identity(nc, ident16)

        # ---- parameters into contraction-major layouts ----
        WaT = [singles.tile([128, A], f32, tag=f"WaT{rt}", name=f"WaT{rt}")
               for rt in range(4)]
        for at in range(2):
            wa_nat = ldpool.tile([128, R], f32, tag="ld", name="ld")
            nc.sync.dma_start(out=wa_nat, in_=Wa[at * 128:(at + 1) * 128, :])
            for rt in range(4):
                ps = ps_sm.tile([128, 128], f32, tag="sm", name="sm")
                nc.tensor.transpose(ps, wa_nat[:, rt * 128:(rt + 1) * 128], ident)
                nc.vector.tensor_copy(out=WaT[rt][:, at * 128:(at + 1) * 128],
                                      in_=ps)
        UaT = [singles.tile([128, A], f16, tag=f"UaT{mh}", name=f"UaT{mh}")
               for mh in range(2)]
        for at in range(2):
            ua_nat = ldpool.tile([128, M], f32, tag="ld", name="ld")
            nc.sync.dma_start(out=ua_nat, in_=Ua[at * 128:(at + 1) * 128, :])
            for mh in range(2):
                ps = ps_sm.tile([128, 128], f32, tag="sm", name="sm")
                nc.tensor.transpose(ps, ua_nat[:, mh * 128:(mh + 1) * 128], ident)
                nc.vector.tensor_copy(out=UaT[mh][:, at * 128:(at + 1) * 128],
                                      in_=ps)
        w_sb = [singles.tile([128, 1], f16, tag=f"w{ah}", name=f"w{ah}")
                for ah in range(2)]
        for ah in range(2):
            nc.gpsimd.dma_start(out=w_sb[ah], in_=w[ah * 128:(ah + 1) * 128, :])

        hsT = [singles.tile([128, BLOC], f32, tag=f"hsT{rt}", name=f"hsT{rt}")
               for rt in range(4)]
        for rt in range(4):
            src = bass.AP(tensor=hs.tensor, offset=rt * 128,
                          ap=[[1, 128], [R, BLOC]])
            nc.sync.dma_start(out=hsT[rt], in_=src)

        q_ps = ps_sm.tile([BLOC, A], f32, tag="sm", name="sm")
        for rt in range(4):
            nc.tensor.matmul(q_ps, lhsT=hsT[rt], rhs=WaT[rt],
                             start=(rt == 0), stop=(rt == 3))
        ba_b = singles.tile([BLOC, A], f32, tag="ba", name="ba")
        nc.sync.dma_start(out=ba_b,
                          in_=bass.AP(tensor=ba.tensor, offset=0,
                                      ap=[[0, BLOC], [1, A]]))
        q_sb = singles.tile([BLOC, A], f32, tag="q", name="q")
        nc.vector.tensor_add(q_sb, q_ps, ba_b)
        qT = [singles.tile([128, BLOC], f32, tag=f"qT{ah}", name=f"qT{ah}")
              for ah in range(2)]
        for ah in range(2):
            ps = ps_sm.tile([128, BLOC], f32, tag="sm", name="sm")
            nc.tensor.transpose(ps, q_sb[:, ah * 128:(ah + 1) * 128],
                                ident[:BLOC, :BLOC])
            nc.vector.tensor_copy(out=qT[ah], in_=ps)

        # ---- flat per-row pipeline ----
        st = {}  # b -> state dict

        def emit_load(b):
            if b == 0:
                fvt = fvt0
            else:
                fvt = fvpool.tile([128, 2, L], f16, tag="fvt", name="fvt")
                src = bass.AP(tensor=fvt_d.tensor, offset=b * M * L,
                              ap=[[L, 128], [128 * L, 2], [1, L]])
                nc.gpsimd.dma_start(out=fvt, in_=src)
            st[b] = {"fvt": fvt, "t": {}}
            st[b]["e_sb"] = small.tile([1, L], f32, tag="e_sb", name="e_sb")
            st[b]["e_d"] = dram.tile([L], f32, tag="e_d", name="e_d")

        def emit_k(b, jg):
            s = st[b]
            for ah in range(2):
                psk = ps_k.tile([128, 2, JW2], f32, tag="psk", name="psk")
                for mh in range(2):
                    for c in range(2):
                        lo = jg * JGW + c * JW2
                        nc.tensor.matmul(
                            psk[:, c, :],
                            lhsT=UaT[mh][:, ah * 128:(ah + 1) * 128],
                            rhs=s["fvt"][:, mh, lo:lo + JW2],
                            start=(mh == 0), stop=(mh == 1))
                t_sb = tpool.tile([128, 2, JW2], f16, tag=f"t{ah}",
                                  name=f"t{ah}")
                nc.scalar.activation(out=t_sb, in_=psk, func=AF.Tanh,
                                     bias=qT[ah][:, b:b + 1], scale=1.0)
                s["t"].setdefault(jg, []).append(t_sb)

        def emit_e(b, jg):
            # w-stationary e rows for jG, evacuated on Act into e_sb
            s = st[b]
            ts = s["t"].pop(jg)
            for c in range(2):
                pse = ps_e.tile([1, JW2], f32, tag="ee", name="ee")
                for ah in range(2):
                    nc.tensor.matmul(pse, lhsT=w_sb[ah], rhs=ts[ah][:, c, :],
                                     start=(ah == 0), stop=(ah == 1))
                lo = jg * JGW + c * JW2
                nc.scalar.copy(out=s["e_sb"][:, lo:lo + JW2], in_=pse)

        def emit_bounce(b):
            s = st[b]
            nc.sync.dma_start(
                out=bass.AP(tensor=s["e_d"].tensor, offset=s["e_d"].offset,
                            ap=[[0, 1], [1, L]]),
                in_=s["e_sb"])
            e_t = small.tile([128, NL], f32, tag="e_t", name="e_t")
            nc.sync.dma_start(
                out=e_t,
                in_=bass.AP(tensor=s["e_d"].tensor, offset=s["e_d"].offset,
                            ap=[[1, 128], [128, NL]]))
            s["e_t"] = e_t

        def emit_softmax_head(b):
            # rmax (DVE) -> ARmax (gps) -> negm (gps) -> exp (Act) -> ARsum
            s = st[b]
            mrow = small.tile([128, 1], f32, tag="mrow", name="mrow")
            nc.vector.reduce_max(out=mrow, in_=s["e_t"],
                                 axis=mybir.AxisListType.X)
            mall = small.tile([128, 1], f32, tag="mall", name="mall")
            nc.gpsimd.partition_all_reduce(mall, mrow, channels=128,
                                           reduce_op=bass_isa.ReduceOp.max)
            negm = small.tile([128, 1], f32, tag="negm", name="negm")
            nc.gpsimd.tensor_scalar_mul(negm, mall, -1.0)
            p_t = ptpool.tile([128, NL], f16, tag="p_t", name="p_t")
            srow = small.tile([128, 1], f32, tag="srow", name="srow")
            nc.scalar.activation(out=p_t, in_=s["e_t"], func=AF.Exp, bias=negm,
                                 scale=1.0, accum_out=srow)
            sall = small.tile([128, 1], f32, tag="sall", name="sall")
            nc.gpsimd.partition_all_reduce(sall, srow, channels=128,
                                           reduce_op=bass_isa.ReduceOp.add)
            s["p_t"] = p_t
            s["sall"] = sall

        def emit_ptranspose(b):
            s = st[b]
            pT_ps = ps_sm.tile([32, 128], f16, tag="sm", name="sm")
            nc.tensor.transpose(pT_ps, s["p_t"], ident16)
            s["pT_ps"] = pT_ps

        def emit_pchain_tail(b):
            # DVE copy out of PSUM, then SWDGE bounce + broadcast, recip
            s = st[b]
            pT_sb = small.tile([32, 128], f16, tag="ptsb", name="ptsb")
            nc.vector.tensor_copy(out=pT_sb, in_=s.pop("pT_ps"))
            p_d = dram.tile([L], f16, tag="p_d", name="p_d")
            nc.gpsimd.dma_start(
                out=bass.AP(tensor=p_d.tensor, offset=p_d.offset,
                            ap=[[128, 32], [1, 128]]),
                in_=pT_sb)
            p_bc = bcpool.tile([128, L], f16, tag="p_bc", name="p_bc")
            nc.gpsimd.dma_start(
                out=p_bc,
                in_=bass.AP(tensor=p_d.tensor, offset=p_d.offset,
                            ap=[[0, 128], [1, L]]))
            s["p_bc"] = p_bc
            rz = small.tile([128, 1], f32, tag="rz", name="rz")
            nc.vector.reciprocal(out=rz, in_=s["sall"])
            s["rz"] = rz

        def emit_ws(b):
            s = st.pop(b)
            for mh in range(2):
                prod = trashp.tile([128, L], f16, tag="trash", name="trash")
                nc.vector.tensor_mul(prod, s["p_bc"], s["fvt"][:, mh, :])
                s1 = trashp.tile([128, L // 2], f16, tag="s1", name="s1")
                nc.vector.tensor_add(s1, prod[:, :L // 2], prod[:, L // 2:])
                s2 = trashp.tile([128, L // 4], f16, tag="s2", name="s2")
                nc.vector.tensor_add(s2, s1[:, :L // 4], s1[:, L // 4:])
                s3 = trashp.tile([128, L // 8], f16, tag="s3", name="s3")
                nc.vector.tensor_add(s3, s2[:, :L // 8], s2[:, L // 8:])
                ctxh = small.tile([128, 1], f32, tag=f"ctxh{mh}",
                                  name=f"ctxh{mh}")
                nc.vector.tensor_reduce(out=ctxh, in_=s3,
                                        axis=mybir.AxisListType.X, op=ALU.add)
                ctxs = small.tile([128, 1], f32, tag=f"ctxs{mh}",
                                  name=f"ctxs{mh}")
                nc.vector.tensor_mul(ctxs, ctxh, s["rz"])
                nc.sync.dma_start(
                    out=bass.AP(tensor=ctx_out.tensor, offset=b * M + mh * 128,
                                ap=[[1, 128], [0, 1]]),
                    in_=ctxs)

        for b in range(BLOC):
            if b == 0:
                emit_load(0)
            if b + 1 < BLOC:
                emit_load(b + 1)  # trigger one row ahead of use
            emit_k(b, 0)
            if b >= 1:
                emit_e(b - 1, 2)
            emit_k(b, 1)
            if b >= 1:
                emit_e(b - 1, 3)
                emit_bounce(b - 1)
                emit_softmax_head(b - 1)
            emit_k(b, 2)
            emit_e(b, 0)
            emit_k(b, 3)
            emit_e(b, 1)
            if b >= 1:
                emit_ptranspose(b - 1)
            if b >= 2:
                emit_ws(b - 2)
            if b >= 1:
                emit_pchain_tail(b - 1)
        # epilogue: row 7 tail + last two weighted sums
        bl = BLOC - 1
        emit_e(bl, 2)
        emit_e(bl, 3)
        emit_bounce(bl)
        emit_softmax_head(bl)
        emit_ptranspose(bl)
        emit_ws(bl - 1)
        emit_pchain_tail(bl)
        emit_ws(bl)

    nc.compile()
    return nc


def _get_nc():
    if "nc" not in _CACHE:
        _CACHE["nc"] = _build()
    return _CACHE["nc"]


def make_in_maps(inputs):
    """Per-core input dicts for run_bass_kernel_spmd (host-side shard +
    fp16 pre-transpose of feature_vectors)."""
    fv = np.asarray(inputs["feature_vectors"])
    fvT = fv.transpose(0, 2, 1).astype(np.float16)  # [B, M, L] fp16
    hs = np.ascontiguousarray(np.asarray(inputs["hidden_state"]),
                              dtype=np.float32)
    params = {
        "Wa": np.ascontiguousarray(np.asarray(inputs["Wa"]), dtype=np.float32),
        "Ua": np.ascontiguousarray(np.asarray(inputs["Ua"]), dtype=np.float32),
        "w": np.ascontiguousarray(np.asarray(inputs["w"]), dtype=np.float32),
        "ba": np.ascontiguousarray(np.asarray(inputs["ba"]), dtype=np.float32),
    }
    return [
        {
            "hidden_state": hs[c * BLOC:(c + 1) * BLOC],
            "fvT": np.ascontiguousarray(fvT[c * BLOC:(c + 1) * BLOC]),
            **params,
        }
        for c in range(NCORES)
    ]


def kernel(hidden_state, feature_vectors, Wa, Ua, w, ba):
    from concourse.bass_utils import run_bass_kernel_spmd

    nc = _get_nc()
    in_maps = make_in_maps({
        "hidden_state": hidden_state,
        "feature_vectors": feature_vectors,
        "Wa": Wa, "Ua": Ua, "w": w, "ba": ba,
    })
    res = run_bass_kernel_spmd(nc, in_maps, list(range(NCORES)))
    return np.concatenate([res.results[c]["context"] for c in range(NCORES)],
                          axis=0)
